# revision 10
# baseline (speedup 1.0000x reference)
"""Causal masked attention (B=8, S=2048, d_model=1024, d_k=d_v=512) on 8 TRN2
NeuronCores, data-parallel over batch (one batch element per core).

Per-core dataflow (all matmuls bf16 with fp32 PSUM accumulation):
  x_q, x_kv f32 --SWDGE cast DMA (row blocks)--> bf16 DRAM scratch
  --DMA transpose--> xT tiles [dm, s] in SBUF.
  qT = Wq^T x_q^T, kT = Wk^T x_kv^T   ([d_k, S] bf16, PE)
  v  = x_kv Wv                        ([S, d_v] bf16, PE)
  scores^T blocks [keys 128, q 512] = kT_chunk^T @ qT (causal-skipped)
  p^T = exp(scale*s^T + kv_bias)      (ACT, kv padding folded into bias)
  boundary blocks *= causal 0/1 tile  (DVE)
  out = p^T.T @ v, den = p^T.T @ 1    (PE), out *= qvalid/den (ACT w/ scale AP)

Fully-masked rows give den==0 -> clamped to 1e-30 -> out = 0 (matches the
reference's NaN->0). Query-padded rows are zeroed via qvalid.

Phases are interleaved per 512-row block so attention compute overlaps the
projection phase's DMA traffic (cast + transpose + weight loads).
"""

import numpy as np
import ml_dtypes
from contextlib import ExitStack

import concourse.bass as bass
import concourse.tile as tile
import concourse.mybir as mybir
from concourse import bacc
from concourse.bass_utils import run_bass_kernel_spmd

B, S, DM, DK, DV = 8, 2048, 1024, 512, 512
NCORES = 8
P = 128
NQJ = S // 512          # 4 query column-blocks of 512
NKC = S // P            # 16 key chunks of 128
NDMC = DM // P          # 8 d_model chunks
NDKC = DK // P          # 4 d_k chunks
SCALE = float(DK) ** -0.5

F32 = mybir.dt.float32
BF16 = mybir.dt.bfloat16
U8 = mybir.dt.uint8
ts = bass.ts


def _emit(nc):
    xq = nc.declare_dram_parameter("xq", [S, DM], F32, isOutput=False)
    xkv = nc.declare_dram_parameter("xkv", [S, DM], F32, isOutput=False)
    wq = nc.declare_dram_parameter("wq", [DM, DK], F32, isOutput=False)
    wk = nc.declare_dram_parameter("wk", [DM, DK], F32, isOutput=False)
    wv = nc.declare_dram_parameter("wv", [DM, DV], F32, isOutput=False)
    qpad = nc.declare_dram_parameter("qpad", [S], U8, isOutput=False)
    kvpad = nc.declare_dram_parameter("kvpad", [S], U8, isOutput=False)
    out = nc.declare_dram_parameter("out", [S, DV], F32, isOutput=True)

    # bf16 DRAM scratch, produced row-block by row-block via HWDGE f32 load ->
    # on-chip cast -> HWDGE bf16 store (the SWDGE HBM->HBM cast path runs at
    # ~half DMA rate, so the cast happens on compute engines instead).
    # chunk-major scratch: each [512,128] transpose source block is fully
    # contiguous, so the xbar M2S side concats 16 rows -> 4KB descriptors.
    xqbf = nc.dram_tensor("xqbf", [NDMC, S, P], BF16)
    xkvbf = nc.dram_tensor("xkvbf", [NDMC, S, P], BF16)

    with ExitStack() as ctx:
        tc = ctx.enter_context(tile.TileContext(nc))
        cst = ctx.enter_context(tc.tile_pool(name="cst", bufs=1))
        xtp = ctx.enter_context(tc.tile_pool(name="xtp", bufs=16))
        ptp = ctx.enter_context(tc.tile_pool(name="ptp", bufs=2))
        etp = ctx.enter_context(tc.tile_pool(name="etp", bufs=2))
        obp = ctx.enter_context(tc.tile_pool(name="obp", bufs=3))
        sml = ctx.enter_context(tc.tile_pool(name="sml", bufs=4))
        psm = ctx.enter_context(tc.tile_pool(name="psm", bufs=4, space="PSUM"))
        psv = ctx.enter_context(tc.tile_pool(name="psv", bufs=2, space="PSUM"))
        psd = ctx.enter_context(tc.tile_pool(name="psd", bufs=2, space="PSUM"))

        stg = ctx.enter_context(tc.tile_pool(name="stg", bufs=2))

        # ---- weights: f32 load (scalar HWDGE) + DVE cast to bf16 ----------
        wqt = cst.tile([P, NDMC, DK], BF16, tag="wqt")
        wkt = cst.tile([P, NDMC, DK], BF16, tag="wkt")
        wvt = cst.tile([P, NDMC, DV], BF16, tag="wvt")

        def w_load(dst, w, nm):
            wf = stg.tile([P, NDMC, DK], F32, tag="wstg", name=nm, bufs=1)
            nc.scalar.dma_start(wf[:], w.rearrange("(c p) n -> p c n", p=P))
            nc.vector.tensor_copy(dst[:], wf[:])

        # ---- x casts: SWDGE DRAM->DRAM per d_model chunk, xq first --------
        w_load(wqt, wq, "wq")
        w_load(wkt, wk, "wk")
        w_load(wvt, wv, "wv")
        for c in range(NDMC):
            nc.gpsimd.dma_start(xqbf[c], xq[:, ts(c, P)])
        for c in range(NDMC):
            nc.gpsimd.dma_start(xkvbf[c], xkv[:, ts(c, P)])

        # ---- constants / masks --------------------------------------------
        ones = cst.tile([P, 1], BF16, tag="ones")
        nc.gpsimd.memset(ones[:], 1.0)

        # causal[k, r, q] = 1.0 where q >= k + 128*r else 0  (4 offsets)
        causal = cst.tile([P, 4, 512], BF16, tag="causal")
        for r in range(4):
            nc.gpsimd.memset(causal[:, r, :], 1.0)
            nc.gpsimd.affine_select(
                out=causal[:, r, :],
                in_=causal[:, r, :],
                compare_op=mybir.AluOpType.is_ge,
                fill=0.0,
                base=-128 * r,
                pattern=[[1, 512]],
                channel_multiplier=-1,
            )

        # padding masks -> per-partition layout [128, 16] (p, chunk)
        ku8 = cst.tile([P, NKC], U8, tag="ku8")
        qu8 = cst.tile([P, NKC], U8, tag="qu8")
        nc.gpsimd.dma_start(ku8[:], kvpad.rearrange("(c p) -> p c", p=P))
        nc.gpsimd.dma_start(qu8[:], qpad.rearrange("(c p) -> p c", p=P))
        kf = cst.tile([P, NKC], F32, tag="kf")
        qf = cst.tile([P, NKC], F32, tag="qf")
        nc.vector.tensor_copy(kf[:], ku8[:])
        nc.vector.tensor_copy(qf[:], qu8[:])
        kvbias = cst.tile([P, NKC], F32, tag="kvbias")
        nc.vector.tensor_scalar_mul(kvbias[:], kf[:], -1e9)
        qvalid = cst.tile([P, NKC], F32, tag="qvalid")
        nc.vector.tensor_scalar(
            qvalid[:], qf[:], -1.0, 1.0,
            mybir.AluOpType.mult, mybir.AluOpType.add,
        )

        # ---- persistent projection outputs --------------------------------
        qT = cst.tile([P, NDKC, S], BF16, tag="qT")     # [dk, s]
        kT = cst.tile([P, NDKC, S], BF16, tag="kT")     # [dk, s]
        vS = cst.tile([P, NKC, DV], BF16, tag="vS")     # [s, dv]

        def xt_load(src, c, sb, nm):
            t = xtp.tile([P, 512], BF16, tag="xt", name=nm)
            nc.sync.dma_start(t[:], src[c, ts(sb, 512), :], transpose=True)
            return t

        def proj_q(sb):
            acc = [psm.tile([P, 512], F32, tag="mm", name=f"qacc{sb}_{i}")
                   for i in range(NDKC)]
            for c in range(NDMC):
                xt = xt_load(xqbf, c, sb, f"xtq{sb}_{c}")
                for d in range(NDKC):
                    nc.tensor.matmul(
                        acc[d][:], wqt[:, c, ts(d, P)], xt[:],
                        start=(c == 0), stop=(c == NDMC - 1),
                    )
            for d in range(NDKC):
                nc.vector.tensor_copy(qT[:, d, ts(sb, 512)], acc[d][:])

        def proj_kv(sb):
            xts = [xt_load(xkvbf, c, sb, f"xtkv{sb}_{c}") for c in range(NDMC)]
            acc = [psm.tile([P, 512], F32, tag="mm", name=f"kacc{sb}_{i}")
                   for i in range(NDKC)]
            for c in range(NDMC):
                for d in range(NDKC):
                    nc.tensor.matmul(
                        acc[d][:], wkt[:, c, ts(d, P)], xts[c][:],
                        start=(c == 0), stop=(c == NDMC - 1),
                    )
            for d in range(NDKC):
                nc.vector.tensor_copy(kT[:, d, ts(sb, 512)], acc[d][:])
            vacc = [psm.tile([P, 512], F32, tag="mm", name=f"vacc{sb}_{i}")
                    for i in range(4)]
            for c in range(NDMC):
                for u in range(4):
                    nc.tensor.matmul(
                        vacc[u][:], xts[c][:, ts(u, P)], wvt[:, c, :],
                        start=(c == 0), stop=(c == NDMC - 1),
                    )
            for u in range(4):
                nc.vector.tensor_copy(vS[:, sb * 4 + u, :], vacc[u][:])

        pts = {}

        def scores(qj):
            nkc = 4 * qj + 4
            pt = ptp.tile([P, NKC, 512], BF16, tag="pt", name=f"pt{qj}")
            pts[qj] = pt
            for kc in range(nkc):
                sp = psm.tile([P, 512], F32, tag="mm", name=f"sp{qj}_{kc}")
                for d in range(NDKC):
                    nc.tensor.matmul(
                        sp[:], kT[:, d, ts(kc, P)], qT[:, d, ts(qj, 512)],
                        start=(d == 0), stop=(d == NDKC - 1),
                    )
                r = kc - 4 * qj
                if r < 0:
                    nc.scalar.activation(
                        pt[:, kc, :], sp[:], mybir.ActivationFunctionType.Exp,
                        bias=kvbias[:, kc:kc + 1], scale=SCALE,
                    )
                else:
                    et = etp.tile([P, 512], BF16, tag="et", name=f"et{qj}_{kc}")
                    nc.scalar.activation(
                        et[:], sp[:], mybir.ActivationFunctionType.Exp,
                        bias=kvbias[:, kc:kc + 1], scale=SCALE,
                    )
                    nc.vector.tensor_mul(pt[:, kc, :], et[:], causal[:, r, :])

        def pv(qb):
            qj = qb // 4
            pt = pts[qj]
            po = psv.tile([P, DV], F32, tag="pv", name=f"po{qb}")
            pd = psd.tile([P, 1], F32, tag="den", name=f"pd{qb}")
            for kc in range(qb + 1):
                lhs = pt[:, kc, ts(qb % 4, P)]
                nc.tensor.matmul(po[:], lhs, vS[:, kc, :],
                                 start=(kc == 0), stop=(kc == qb))
                nc.tensor.matmul(pd[:], lhs, ones[:],
                                 start=(kc == 0), stop=(kc == qb))
            den = sml.tile([P, 1], F32, tag="den_s", name=f"den{qb}")
            nc.vector.tensor_scalar_max(den[:], pd[:], 1e-30)
            rec = sml.tile([P, 1], F32, tag="rec", name=f"rec{qb}")
            nc.vector.reciprocal(rec[:], den[:])
            sc = sml.tile([P, 1], F32, tag="sc", name=f"sc{qb}")
            nc.vector.tensor_scalar_mul(sc[:], rec[:], qvalid[:, qb:qb + 1])
            ob = obp.tile([P, DV], F32, tag="ob", name=f"ob{qb}")
            nc.scalar.mul(ob[:], po[:], sc[:])
            nc.scalar.dma_start(out[ts(qb, P), :], ob[:])

        # schedule: all qT first (rides the xq cast stream), then kv
        # projections interleaved with attention so the PE stays dense.
        for sb in range(NQJ):
            proj_q(sb)
        proj_kv(0); scores(0)
        proj_kv(1); scores(1)
        for qb in range(0, 4):
            pv(qb)
        proj_kv(2); scores(2)
        for qb in range(4, 8):
            pv(qb)
        proj_kv(3); scores(3)
        for qb in range(8, 16):
            pv(qb)

    nc.compile()
    return nc


_NC_CACHE = []


def _get_nc():
    if not _NC_CACHE:
        nc = bacc.Bacc("TRN2")
        _NC_CACHE.append(_emit(nc))
    return _NC_CACHE[0]


def _in_maps(inputs):
    sq = np.ascontiguousarray(np.asarray(inputs["source_query"], dtype=np.float32))
    skv = np.ascontiguousarray(np.asarray(inputs["source_key_value"], dtype=np.float32))
    qp = np.asarray(inputs["source_query_padding_mask"]).astype(np.uint8)
    kvp = np.asarray(inputs["source_key_value_padding_mask"]).astype(np.uint8)
    Wq = np.ascontiguousarray(np.asarray(inputs["Wq"], dtype=np.float32))
    Wk = np.ascontiguousarray(np.asarray(inputs["Wk"], dtype=np.float32))
    Wv = np.ascontiguousarray(np.asarray(inputs["Wv"], dtype=np.float32))
    maps = []
    for b in range(NCORES):
        maps.append({
            "xq": sq[b], "xkv": skv[b],
            "wq": Wq, "wk": Wk, "wv": Wv,
            "qpad": np.ascontiguousarray(qp[b]),
            "kvpad": np.ascontiguousarray(kvp[b]),
        })
    return maps


def _execute(inputs, **kw):
    nc = _get_nc()
    res = run_bass_kernel_spmd(nc, _in_maps(inputs), core_ids=list(range(NCORES)), **kw)
    outs = np.stack([res.results[b]["out"] for b in range(NCORES)], axis=0)
    return outs.astype(np.float32), res


def kernel(**inputs) -> np.ndarray:
    out, _ = _execute(inputs)
    return out


# revision 11
# speedup vs baseline: 1.2373x; 1.2373x over previous
"""Causal masked attention (B=8, S=2048, d_model=1024, d_k=d_v=512) on 8 TRN2
NeuronCores, data-parallel over batch (one batch element per core).

Per-core dataflow (all matmuls bf16 with fp32 PSUM accumulation):
  x_q, x_kv f32 --SWDGE cast DMA (row blocks)--> bf16 DRAM scratch
  --DMA transpose--> xT tiles [dm, s] in SBUF.
  qT = Wq^T x_q^T, kT = Wk^T x_kv^T   ([d_k, S] bf16, PE)
  v  = x_kv Wv                        ([S, d_v] bf16, PE)
  scores^T blocks [keys 128, q 512] = kT_chunk^T @ qT (causal-skipped)
  p^T = exp(scale*s^T + kv_bias)      (ACT, kv padding folded into bias)
  boundary blocks *= causal 0/1 tile  (DVE)
  out = p^T.T @ v, den = p^T.T @ 1    (PE), out *= qvalid/den (ACT w/ scale AP)

Fully-masked rows give den==0 -> clamped to 1e-30 -> out = 0 (matches the
reference's NaN->0). Query-padded rows are zeroed via qvalid.

Phases are interleaved per 512-row block so attention compute overlaps the
projection phase's DMA traffic (cast + transpose + weight loads).
"""

import numpy as np
import ml_dtypes
from contextlib import ExitStack

import concourse.bass as bass
import concourse.tile as tile
import concourse.mybir as mybir
from concourse import bacc
from concourse.bass_utils import run_bass_kernel_spmd

B, S, DM, DK, DV = 8, 2048, 1024, 512, 512
NCORES = 8
P = 128
NQJ = S // 512          # 4 query column-blocks of 512
NKC = S // P            # 16 key chunks of 128
NDMC = DM // P          # 8 d_model chunks
NDKC = DK // P          # 4 d_k chunks
SCALE = float(DK) ** -0.5

F32 = mybir.dt.float32
BF16 = mybir.dt.bfloat16
U8 = mybir.dt.uint8
ts = bass.ts


def _emit(nc):
    xq = nc.declare_dram_parameter("xq", [S, DM], F32, isOutput=False)
    xkv = nc.declare_dram_parameter("xkv", [S, DM], F32, isOutput=False)
    wq = nc.declare_dram_parameter("wq", [DM, DK], F32, isOutput=False)
    wk = nc.declare_dram_parameter("wk", [DM, DK], F32, isOutput=False)
    wv = nc.declare_dram_parameter("wv", [DM, DV], F32, isOutput=False)
    qpad = nc.declare_dram_parameter("qpad", [S], U8, isOutput=False)
    kvpad = nc.declare_dram_parameter("kvpad", [S], U8, isOutput=False)
    out = nc.declare_dram_parameter("out", [S, DV], F32, isOutput=True)

    # bf16 DRAM scratch, produced row-block by row-block via HWDGE f32 load ->
    # on-chip cast -> HWDGE bf16 store (the SWDGE HBM->HBM cast path runs at
    # ~half DMA rate, so the cast happens on compute engines instead).
    # row-major scratch: the cast DMA runs with large contiguous descriptors;
    # transposes read strided [1024,128] slabs (fast at >=256KB grain).
    xqbf = nc.dram_tensor("xqbf", [S, DM], BF16)
    xkvbf = nc.dram_tensor("xkvbf", [S, DM], BF16)

    with ExitStack() as ctx:
        tc = ctx.enter_context(tile.TileContext(nc))
        cst = ctx.enter_context(tc.tile_pool(name="cst", bufs=1))
        xtp = ctx.enter_context(tc.tile_pool(name="xtp", bufs=16))  # [128,1024] bf16 x16 = 4MB
        ptp = ctx.enter_context(tc.tile_pool(name="ptp", bufs=2))
        etp = ctx.enter_context(tc.tile_pool(name="etp", bufs=2))
        obp = ctx.enter_context(tc.tile_pool(name="obp", bufs=3))
        sml = ctx.enter_context(tc.tile_pool(name="sml", bufs=4))
        psm = ctx.enter_context(tc.tile_pool(name="psm", bufs=4, space="PSUM"))
        psv = ctx.enter_context(tc.tile_pool(name="psv", bufs=2, space="PSUM"))
        psd = ctx.enter_context(tc.tile_pool(name="psd", bufs=2, space="PSUM"))

        stg = ctx.enter_context(tc.tile_pool(name="stg", bufs=2))

        # ---- weights: f32 load (scalar HWDGE) + DVE cast to bf16 ----------
        wqt = cst.tile([P, NDMC, DK], BF16, tag="wqt")
        wkt = cst.tile([P, NDMC, DK], BF16, tag="wkt")
        wvt = cst.tile([P, NDMC, DV], BF16, tag="wvt")

        def w_load(dst, w, nm):
            wf = stg.tile([P, NDMC, DK], F32, tag="wstg", name=nm, bufs=1)
            nc.scalar.dma_start(wf[:], w.rearrange("(c p) n -> p c n", p=P))
            nc.vector.tensor_copy(dst[:], wf[:])

        # ---- x casts: SWDGE DRAM->DRAM row blocks, interleaved xq/xkv -----
        w_load(wqt, wq, "wq")
        w_load(wkt, wk, "wk")
        w_load(wvt, wv, "wv")
        for h in range(2):
            for r in range(2):
                nc.gpsimd.dma_start(xqbf[ts(2 * h + r, 512), :],
                                    xq[ts(2 * h + r, 512), :])
            for r in range(2):
                nc.gpsimd.dma_start(xkvbf[ts(2 * h + r, 512), :],
                                    xkv[ts(2 * h + r, 512), :])

        # ---- constants / masks --------------------------------------------
        ones = cst.tile([P, 1], BF16, tag="ones")
        nc.gpsimd.memset(ones[:], 1.0)

        # causal[k, r, q] = 1.0 where q >= k + 128*r else 0  (4 offsets)
        causal = cst.tile([P, 4, 512], BF16, tag="causal")
        for r in range(4):
            nc.gpsimd.memset(causal[:, r, :], 1.0)
            nc.gpsimd.affine_select(
                out=causal[:, r, :],
                in_=causal[:, r, :],
                compare_op=mybir.AluOpType.is_ge,
                fill=0.0,
                base=-128 * r,
                pattern=[[1, 512]],
                channel_multiplier=-1,
            )

        # padding masks -> per-partition layout [128, 16] (p, chunk)
        ku8 = cst.tile([P, NKC], U8, tag="ku8")
        qu8 = cst.tile([P, NKC], U8, tag="qu8")
        nc.gpsimd.dma_start(ku8[:], kvpad.rearrange("(c p) -> p c", p=P))
        nc.gpsimd.dma_start(qu8[:], qpad.rearrange("(c p) -> p c", p=P))
        kf = cst.tile([P, NKC], F32, tag="kf")
        qf = cst.tile([P, NKC], F32, tag="qf")
        nc.vector.tensor_copy(kf[:], ku8[:])
        nc.vector.tensor_copy(qf[:], qu8[:])
        kvbias = cst.tile([P, NKC], F32, tag="kvbias")
        nc.vector.tensor_scalar_mul(kvbias[:], kf[:], -1e9)
        qvalid = cst.tile([P, NKC], F32, tag="qvalid")
        nc.vector.tensor_scalar(
            qvalid[:], qf[:], -1.0, 1.0,
            mybir.AluOpType.mult, mybir.AluOpType.add,
        )

        # ---- persistent projection outputs --------------------------------
        qT = cst.tile([P, NDKC, S], BF16, tag="qT")     # [dk, s]
        kT = cst.tile([P, NDKC, S], BF16, tag="kT")     # [dk, s]
        vS = cst.tile([P, NKC, DV], BF16, tag="vS")     # [s, dv]

        xts_cache = {}

        def xt_get(src, key, c, sb):
            h = sb // 2
            if (key, h, c) not in xts_cache:
                t = xtp.tile([P, 1024], BF16, tag="xt", name=f"xt{key}{h}_{c}")
                nc.sync.dma_start(t[:], src[ts(h, 1024), ts(c, P)],
                                  transpose=True)
                xts_cache[(key, h, c)] = t
            return xts_cache[(key, h, c)][:, ts(sb % 2, 512)]

        def proj_q(sb):
            acc = [psm.tile([P, 512], F32, tag="mm", name=f"qacc{sb}_{i}")
                   for i in range(NDKC)]
            for c in range(NDMC):
                xt = xt_get(xqbf, "q", c, sb)
                for d in range(NDKC):
                    nc.tensor.matmul(
                        acc[d][:], wqt[:, c, ts(d, P)], xt,
                        start=(c == 0), stop=(c == NDMC - 1),
                    )
            for d in range(NDKC):
                nc.vector.tensor_copy(qT[:, d, ts(sb, 512)], acc[d][:])

        def proj_kv(sb):
            xts = [xt_get(xkvbf, "kv", c, sb) for c in range(NDMC)]
            acc = [psm.tile([P, 512], F32, tag="mm", name=f"kacc{sb}_{i}")
                   for i in range(NDKC)]
            for c in range(NDMC):
                for d in range(NDKC):
                    nc.tensor.matmul(
                        acc[d][:], wkt[:, c, ts(d, P)], xts[c],
                        start=(c == 0), stop=(c == NDMC - 1),
                    )
            for d in range(NDKC):
                nc.vector.tensor_copy(kT[:, d, ts(sb, 512)], acc[d][:])
            vacc = [psm.tile([P, 512], F32, tag="mm", name=f"vacc{sb}_{i}")
                    for i in range(4)]
            for c in range(NDMC):
                for u in range(4):
                    nc.tensor.matmul(
                        vacc[u][:], xts[c][:, ts(u, P)], wvt[:, c, :],
                        start=(c == 0), stop=(c == NDMC - 1),
                    )
            for u in range(4):
                nc.vector.tensor_copy(vS[:, sb * 4 + u, :], vacc[u][:])

        pts = {}

        def scores(qj):
            nkc = 4 * qj + 4
            pt = ptp.tile([P, NKC, 512], BF16, tag="pt", name=f"pt{qj}")
            pts[qj] = pt
            for kc in range(nkc):
                sp = psm.tile([P, 512], F32, tag="mm", name=f"sp{qj}_{kc}")
                for d in range(NDKC):
                    nc.tensor.matmul(
                        sp[:], kT[:, d, ts(kc, P)], qT[:, d, ts(qj, 512)],
                        start=(d == 0), stop=(d == NDKC - 1),
                    )
                r = kc - 4 * qj
                if r < 0:
                    nc.scalar.activation(
                        pt[:, kc, :], sp[:], mybir.ActivationFunctionType.Exp,
                        bias=kvbias[:, kc:kc + 1], scale=SCALE,
                    )
                else:
                    et = etp.tile([P, 512], BF16, tag="et", name=f"et{qj}_{kc}")
                    nc.scalar.activation(
                        et[:], sp[:], mybir.ActivationFunctionType.Exp,
                        bias=kvbias[:, kc:kc + 1], scale=SCALE,
                    )
                    nc.vector.tensor_mul(pt[:, kc, :], et[:], causal[:, r, :])

        def pv(qb):
            qj = qb // 4
            pt = pts[qj]
            po = psv.tile([P, DV], F32, tag="pv", name=f"po{qb}")
            pd = psd.tile([P, 1], F32, tag="den", name=f"pd{qb}")
            for kc in range(qb + 1):
                lhs = pt[:, kc, ts(qb % 4, P)]
                nc.tensor.matmul(po[:], lhs, vS[:, kc, :],
                                 start=(kc == 0), stop=(kc == qb))
                nc.tensor.matmul(pd[:], lhs, ones[:],
                                 start=(kc == 0), stop=(kc == qb))
            den = sml.tile([P, 1], F32, tag="den_s", name=f"den{qb}")
            nc.vector.tensor_scalar_max(den[:], pd[:], 1e-30)
            rec = sml.tile([P, 1], F32, tag="rec", name=f"rec{qb}")
            nc.vector.reciprocal(rec[:], den[:])
            sc = sml.tile([P, 1], F32, tag="sc", name=f"sc{qb}")
            nc.vector.tensor_scalar_mul(sc[:], rec[:], qvalid[:, qb:qb + 1])
            ob = obp.tile([P, DV], F32, tag="ob", name=f"ob{qb}")
            nc.scalar.mul(ob[:], po[:], sc[:])
            nc.scalar.dma_start(out[ts(qb, P), :], ob[:])

        # schedule by half-tensor: qT rides the xq cast stream, kv + attention
        # backfill while the second halves stream in.
        proj_q(0); proj_q(1)
        proj_kv(0); scores(0)
        proj_kv(1); scores(1)
        for qb in range(0, 4):
            pv(qb)
        proj_q(2); proj_q(3)
        proj_kv(2); scores(2)
        for qb in range(4, 8):
            pv(qb)
        proj_kv(3); scores(3)
        for qb in range(8, 16):
            pv(qb)

    nc.compile()
    return nc


_NC_CACHE = []


def _get_nc():
    if not _NC_CACHE:
        nc = bacc.Bacc("TRN2")
        _NC_CACHE.append(_emit(nc))
    return _NC_CACHE[0]


def _in_maps(inputs):
    sq = np.ascontiguousarray(np.asarray(inputs["source_query"], dtype=np.float32))
    skv = np.ascontiguousarray(np.asarray(inputs["source_key_value"], dtype=np.float32))
    qp = np.asarray(inputs["source_query_padding_mask"]).astype(np.uint8)
    kvp = np.asarray(inputs["source_key_value_padding_mask"]).astype(np.uint8)
    Wq = np.ascontiguousarray(np.asarray(inputs["Wq"], dtype=np.float32))
    Wk = np.ascontiguousarray(np.asarray(inputs["Wk"], dtype=np.float32))
    Wv = np.ascontiguousarray(np.asarray(inputs["Wv"], dtype=np.float32))
    maps = []
    for b in range(NCORES):
        maps.append({
            "xq": sq[b], "xkv": skv[b],
            "wq": Wq, "wk": Wk, "wv": Wv,
            "qpad": np.ascontiguousarray(qp[b]),
            "kvpad": np.ascontiguousarray(kvp[b]),
        })
    return maps


def _execute(inputs, **kw):
    nc = _get_nc()
    res = run_bass_kernel_spmd(nc, _in_maps(inputs), core_ids=list(range(NCORES)), **kw)
    outs = np.stack([res.results[b]["out"] for b in range(NCORES)], axis=0)
    return outs.astype(np.float32), res


def kernel(**inputs) -> np.ndarray:
    out, _ = _execute(inputs)
    return out


# revision 12
# speedup vs baseline: 1.2511x; 1.0112x over previous
"""Causal masked attention (B=8, S=2048, d_model=1024, d_k=d_v=512) on 8 TRN2
NeuronCores, data-parallel over batch (one batch element per core).

Per-core dataflow (all matmuls bf16 with fp32 PSUM accumulation):
  x_q, x_kv f32 --SWDGE cast DMA (row blocks)--> bf16 DRAM scratch
  --DMA transpose--> xT tiles [dm, s] in SBUF.
  qT = Wq^T x_q^T, kT = Wk^T x_kv^T   ([d_k, S] bf16, PE)
  v  = x_kv Wv                        ([S, d_v] bf16, PE)
  scores^T blocks [keys 128, q 512] = kT_chunk^T @ qT (causal-skipped)
  p^T = exp(scale*s^T + kv_bias)      (ACT, kv padding folded into bias)
  boundary blocks *= causal 0/1 tile  (DVE)
  out = p^T.T @ v, den = p^T.T @ 1    (PE), out *= qvalid/den (ACT w/ scale AP)

Fully-masked rows give den==0 -> clamped to 1e-30 -> out = 0 (matches the
reference's NaN->0). Query-padded rows are zeroed via qvalid.

Phases are interleaved per 512-row block so attention compute overlaps the
projection phase's DMA traffic (cast + transpose + weight loads).
"""

import numpy as np
import ml_dtypes
from contextlib import ExitStack

import concourse.bass as bass
import concourse.tile as tile
import concourse.mybir as mybir
from concourse import bacc
from concourse.bass_utils import run_bass_kernel_spmd

B, S, DM, DK, DV = 8, 2048, 1024, 512, 512
NCORES = 8
P = 128
NQJ = S // 512          # 4 query column-blocks of 512
NKC = S // P            # 16 key chunks of 128
NDMC = DM // P          # 8 d_model chunks
NDKC = DK // P          # 4 d_k chunks
SCALE = float(DK) ** -0.5

F32 = mybir.dt.float32
BF16 = mybir.dt.bfloat16
U8 = mybir.dt.uint8
ts = bass.ts


def _emit(nc):
    xq = nc.declare_dram_parameter("xq", [S, DM], F32, isOutput=False)
    xkv = nc.declare_dram_parameter("xkv", [S, DM], F32, isOutput=False)
    wq = nc.declare_dram_parameter("wq", [DM, DK], F32, isOutput=False)
    wk = nc.declare_dram_parameter("wk", [DM, DK], F32, isOutput=False)
    wv = nc.declare_dram_parameter("wv", [DM, DV], F32, isOutput=False)
    qpad = nc.declare_dram_parameter("qpad", [S], U8, isOutput=False)
    kvpad = nc.declare_dram_parameter("kvpad", [S], U8, isOutput=False)
    out = nc.declare_dram_parameter("out", [S, DV], F32, isOutput=True)

    # bf16 DRAM scratch, produced row-block by row-block via HWDGE f32 load ->
    # on-chip cast -> HWDGE bf16 store (the SWDGE HBM->HBM cast path runs at
    # ~half DMA rate, so the cast happens on compute engines instead).
    # row-major scratch: the cast DMA runs with large contiguous descriptors;
    # transposes read strided [1024,128] slabs (fast at >=256KB grain).
    xqbf = nc.dram_tensor("xqbf", [S, DM], BF16)
    xkvbf = nc.dram_tensor("xkvbf", [S, DM], BF16)

    with ExitStack() as ctx:
        tc = ctx.enter_context(tile.TileContext(nc))
        cst = ctx.enter_context(tc.tile_pool(name="cst", bufs=1))
        xtp = ctx.enter_context(tc.tile_pool(name="xtp", bufs=16))  # [128,1024] bf16 x16 = 4MB
        ptp = ctx.enter_context(tc.tile_pool(name="ptp", bufs=2))
        etp = ctx.enter_context(tc.tile_pool(name="etp", bufs=2))
        obp = ctx.enter_context(tc.tile_pool(name="obp", bufs=3))
        sml = ctx.enter_context(tc.tile_pool(name="sml", bufs=4))
        psm = ctx.enter_context(tc.tile_pool(name="psm", bufs=4, space="PSUM"))
        psv = ctx.enter_context(tc.tile_pool(name="psv", bufs=2, space="PSUM"))
        psd = ctx.enter_context(tc.tile_pool(name="psd", bufs=2, space="PSUM"))

        stg = ctx.enter_context(tc.tile_pool(name="stg", bufs=2))

        # ---- weights: f32 load (scalar HWDGE) + DVE cast to bf16 ----------
        wqt = cst.tile([P, NDMC, DK], BF16, tag="wqt")
        wkt = cst.tile([P, NDMC, DK], BF16, tag="wkt")
        wvt = cst.tile([P, NDMC, DV], BF16, tag="wvt")

        def w_load(dst, w, nm):
            wf = stg.tile([P, NDMC, DK], F32, tag="wstg", name=nm, bufs=1)
            nc.scalar.dma_start(wf[:], w.rearrange("(c p) n -> p c n", p=P))
            nc.vector.tensor_copy(dst[:], wf[:])

        # ---- x casts: SWDGE DRAM->DRAM row blocks, interleaved xq/xkv -----
        w_load(wqt, wq, "wq")
        w_load(wkt, wk, "wk")
        w_load(wvt, wv, "wv")
        for h in range(2):
            nc.gpsimd.dma_start(xqbf[ts(h, 1024), :], xq[ts(h, 1024), :])
            nc.gpsimd.dma_start(xkvbf[ts(h, 1024), :], xkv[ts(h, 1024), :])

        # ---- constants / masks --------------------------------------------
        ones = cst.tile([P, 1], BF16, tag="ones")
        nc.gpsimd.memset(ones[:], 1.0)

        # causal[k, r, q] = 1.0 where q >= k + 128*r else 0  (4 offsets)
        causal = cst.tile([P, 4, 512], BF16, tag="causal")
        for r in range(4):
            nc.gpsimd.memset(causal[:, r, :], 1.0)
            nc.gpsimd.affine_select(
                out=causal[:, r, :],
                in_=causal[:, r, :],
                compare_op=mybir.AluOpType.is_ge,
                fill=0.0,
                base=-128 * r,
                pattern=[[1, 512]],
                channel_multiplier=-1,
            )

        # padding masks -> per-partition layout [128, 16] (p, chunk)
        ku8 = cst.tile([P, NKC], U8, tag="ku8")
        qu8 = cst.tile([P, NKC], U8, tag="qu8")
        nc.gpsimd.dma_start(ku8[:], kvpad.rearrange("(c p) -> p c", p=P))
        nc.gpsimd.dma_start(qu8[:], qpad.rearrange("(c p) -> p c", p=P))
        kf = cst.tile([P, NKC], F32, tag="kf")
        qf = cst.tile([P, NKC], F32, tag="qf")
        nc.vector.tensor_copy(kf[:], ku8[:])
        nc.vector.tensor_copy(qf[:], qu8[:])
        kvbias = cst.tile([P, NKC], F32, tag="kvbias")
        nc.vector.tensor_scalar_mul(kvbias[:], kf[:], -1e9)
        qvalid = cst.tile([P, NKC], F32, tag="qvalid")
        nc.vector.tensor_scalar(
            qvalid[:], qf[:], -1.0, 1.0,
            mybir.AluOpType.mult, mybir.AluOpType.add,
        )

        # ---- persistent projection outputs --------------------------------
        qT = cst.tile([P, NDKC, S], BF16, tag="qT")     # [dk, s]
        kT = cst.tile([P, NDKC, S], BF16, tag="kT")     # [dk, s]
        vS = cst.tile([P, NKC, DV], BF16, tag="vS")     # [s, dv]

        xts_cache = {}

        def xt_get(src, key, c, sb):
            h = sb // 2
            if (key, h, c) not in xts_cache:
                t = xtp.tile([P, 1024], BF16, tag="xt", name=f"xt{key}{h}_{c}")
                nc.sync.dma_start(t[:], src[ts(h, 1024), ts(c, P)],
                                  transpose=True)
                xts_cache[(key, h, c)] = t
            return xts_cache[(key, h, c)][:, ts(sb % 2, 512)]

        def proj_q(sb):
            acc = [psm.tile([P, 512], F32, tag="mm", name=f"qacc{sb}_{i}")
                   for i in range(NDKC)]
            for c in range(NDMC):
                xt = xt_get(xqbf, "q", c, sb)
                for d in range(NDKC):
                    nc.tensor.matmul(
                        acc[d][:], wqt[:, c, ts(d, P)], xt,
                        start=(c == 0), stop=(c == NDMC - 1),
                    )
            for d in range(NDKC):
                nc.vector.tensor_copy(qT[:, d, ts(sb, 512)], acc[d][:])

        def proj_kv(sb):
            xts = [xt_get(xkvbf, "kv", c, sb) for c in range(NDMC)]
            acc = [psm.tile([P, 512], F32, tag="mm", name=f"kacc{sb}_{i}")
                   for i in range(NDKC)]
            for c in range(NDMC):
                for d in range(NDKC):
                    nc.tensor.matmul(
                        acc[d][:], wkt[:, c, ts(d, P)], xts[c],
                        start=(c == 0), stop=(c == NDMC - 1),
                    )
            for d in range(NDKC):
                nc.vector.tensor_copy(kT[:, d, ts(sb, 512)], acc[d][:])
            vacc = [psm.tile([P, 512], F32, tag="mm", name=f"vacc{sb}_{i}")
                    for i in range(4)]
            for c in range(NDMC):
                for u in range(4):
                    nc.tensor.matmul(
                        vacc[u][:], xts[c][:, ts(u, P)], wvt[:, c, :],
                        start=(c == 0), stop=(c == NDMC - 1),
                    )
            for u in range(4):
                nc.vector.tensor_copy(vS[:, sb * 4 + u, :], vacc[u][:])

        pts = {}

        def scores(qj):
            nkc = 4 * qj + 4
            pt = ptp.tile([P, NKC, 512], BF16, tag="pt", name=f"pt{qj}")
            pts[qj] = pt
            for kc in range(nkc):
                sp = psm.tile([P, 512], F32, tag="mm", name=f"sp{qj}_{kc}")
                for d in range(NDKC):
                    nc.tensor.matmul(
                        sp[:], kT[:, d, ts(kc, P)], qT[:, d, ts(qj, 512)],
                        start=(d == 0), stop=(d == NDKC - 1),
                    )
                r = kc - 4 * qj
                if r < 0:
                    nc.scalar.activation(
                        pt[:, kc, :], sp[:], mybir.ActivationFunctionType.Exp,
                        bias=kvbias[:, kc:kc + 1], scale=SCALE,
                    )
                else:
                    et = etp.tile([P, 512], BF16, tag="et", name=f"et{qj}_{kc}")
                    nc.scalar.activation(
                        et[:], sp[:], mybir.ActivationFunctionType.Exp,
                        bias=kvbias[:, kc:kc + 1], scale=SCALE,
                    )
                    nc.vector.tensor_mul(pt[:, kc, :], et[:], causal[:, r, :])

        def pv(qb):
            qj = qb // 4
            pt = pts[qj]
            po = psv.tile([P, DV], F32, tag="pv", name=f"po{qb}")
            pd = psd.tile([P, 1], F32, tag="den", name=f"pd{qb}")
            for kc in range(qb + 1):
                lhs = pt[:, kc, ts(qb % 4, P)]
                nc.tensor.matmul(po[:], lhs, vS[:, kc, :],
                                 start=(kc == 0), stop=(kc == qb))
                nc.tensor.matmul(pd[:], lhs, ones[:],
                                 start=(kc == 0), stop=(kc == qb))
            den = sml.tile([P, 1], F32, tag="den_s", name=f"den{qb}")
            nc.vector.tensor_scalar_max(den[:], pd[:], 1e-30)
            rec = sml.tile([P, 1], F32, tag="rec", name=f"rec{qb}")
            nc.vector.reciprocal(rec[:], den[:])
            sc = sml.tile([P, 1], F32, tag="sc", name=f"sc{qb}")
            nc.vector.tensor_scalar_mul(sc[:], rec[:], qvalid[:, qb:qb + 1])
            ob = obp.tile([P, DV], F32, tag="ob", name=f"ob{qb}")
            nc.scalar.mul(ob[:], po[:], sc[:])
            nc.scalar.dma_start(out[ts(qb, P), :], ob[:])

        # schedule by half-tensor: qT rides the xq cast stream, kv + attention
        # backfill while the second halves stream in.
        proj_q(0); proj_q(1)
        proj_kv(0); scores(0)
        proj_kv(1); scores(1)
        for qb in range(0, 4):
            pv(qb)
        proj_q(2); proj_q(3)
        proj_kv(2); scores(2)
        for qb in range(4, 8):
            pv(qb)
        proj_kv(3); scores(3)
        for qb in range(8, 16):
            pv(qb)

    nc.compile()
    return nc


_NC_CACHE = []


def _get_nc():
    if not _NC_CACHE:
        nc = bacc.Bacc("TRN2")
        _NC_CACHE.append(_emit(nc))
    return _NC_CACHE[0]


def _in_maps(inputs):
    sq = np.ascontiguousarray(np.asarray(inputs["source_query"], dtype=np.float32))
    skv = np.ascontiguousarray(np.asarray(inputs["source_key_value"], dtype=np.float32))
    qp = np.asarray(inputs["source_query_padding_mask"]).astype(np.uint8)
    kvp = np.asarray(inputs["source_key_value_padding_mask"]).astype(np.uint8)
    Wq = np.ascontiguousarray(np.asarray(inputs["Wq"], dtype=np.float32))
    Wk = np.ascontiguousarray(np.asarray(inputs["Wk"], dtype=np.float32))
    Wv = np.ascontiguousarray(np.asarray(inputs["Wv"], dtype=np.float32))
    maps = []
    for b in range(NCORES):
        maps.append({
            "xq": sq[b], "xkv": skv[b],
            "wq": Wq, "wk": Wk, "wv": Wv,
            "qpad": np.ascontiguousarray(qp[b]),
            "kvpad": np.ascontiguousarray(kvp[b]),
        })
    return maps


def _execute(inputs, **kw):
    nc = _get_nc()
    res = run_bass_kernel_spmd(nc, _in_maps(inputs), core_ids=list(range(NCORES)), **kw)
    outs = np.stack([res.results[b]["out"] for b in range(NCORES)], axis=0)
    return outs.astype(np.float32), res


def kernel(**inputs) -> np.ndarray:
    out, _ = _execute(inputs)
    return out
